# revision 1
# baseline (speedup 1.0000x reference)
"""DiffPool batched-graph layer on 8 TRN2 NeuronCores (Bass/Tile).

Strategy: shard the B=64 graphs across 8 cores (8 graphs/core). All segment
ops become dense per-graph matmuls against the graph's adjacency-count matrix
A [n, n], which is built on-chip in SBUF by GPSIMD local_scatter from
host-packed (dedup'd, per-src-node padded) edge lists. A^T comes from PE
transposes. The mean-aggregator degree falls out of the same matmul via an
appended ones-column on h. Final h_new/blocks are per-graph [128, 256] tiles;
the host assembles the block-diagonal adj_new.
"""
from contextlib import ExitStack

import numpy as np
import ml_dtypes

import concourse.bass as bass
import concourse.bacc as bacc
import concourse.mybir as mybir
from concourse import tile
from concourse.bass_utils import run_bass_kernel_spmd

bf16 = ml_dtypes.bfloat16
AF = mybir.ActivationFunctionType
dt = mybir.dt

# Problem constants (hardcoded per harness contract)
B, n, F, K, E = 64, 1024, 128, 128, 16384
NCORES = 8
GPC = B // NCORES       # graphs per core
NB = n // 128           # 128-row blocks per graph
EPS = 1e-12
FP = F + 2              # h_aug free: F features + ones col + pad

_compiled = {}          # Spad -> compiled Bacc


# ---------------------------------------------------------------- host prep

def _pack_graph_edges(src, dst, Spad):
    """One graph's edges -> (idx int16 [128, NB, Spad], cnt bf16 same shape).

    Dedup (src,dst) with counts; node u = b*128+p gets its (dst, count) list in
    partition p, block b, padded with -1.
    """
    key = src.astype(np.int64) * n + dst.astype(np.int64)
    uniq, counts = np.unique(key, return_counts=True)
    us = (uniq // n).astype(np.int64)
    vs = (uniq % n).astype(np.int64)
    # slot j within each run of equal us (uniq is sorted so us runs are contiguous)
    starts = np.r_[0, np.flatnonzero(np.diff(us)) + 1]
    run_len = np.diff(np.r_[starts, len(us)])
    j = np.arange(len(us)) - np.repeat(starts, run_len)
    smax = int(j.max()) + 1 if len(j) else 1
    if smax > Spad:
        raise ValueError(f"Spad {Spad} too small, need {smax}")
    idx = np.full((128, NB, Spad), -1, dtype=np.int16)
    cnt = np.zeros((128, NB, Spad), dtype=bf16)
    idx[us % 128, us // 128, j] = vs.astype(np.int16)
    cnt[us % 128, us // 128, j] = counts.astype(bf16)
    return idx, cnt


def _host_prep(h, src_local, dst_local, W_feat, b_feat, W_pool, b_pool):
    h = np.asarray(h, dtype=np.float32)
    src_local = np.asarray(src_local)
    dst_local = np.asarray(dst_local)

    # global Spad (compile-time constant), rounded up to multiple of 4
    deg_max = 0
    packs = []
    # first pass to find Spad cheaply: count max multiplicity-run per src node
    for g in range(B):
        key = src_local[g].astype(np.int64) * n + dst_local[g].astype(np.int64)
        uniq = np.unique(key)
        us = uniq // n
        _, c = np.unique(us, return_counts=True)
        deg_max = max(deg_max, int(c.max()))
    Spad = (deg_max + 3) // 4 * 4

    for g in range(B):
        packs.append(_pack_graph_edges(src_local[g], dst_local[g], Spad))

    hb = h.astype(bf16)                       # [N, F] bf16
    w1 = np.concatenate([W_feat[:F], W_pool[:F]], axis=1).astype(bf16)   # [128, 256]
    w2 = np.concatenate([W_feat[F:], W_pool[F:]], axis=1).astype(bf16)   # [128, 256]
    brow = np.concatenate([b_feat, b_pool]).reshape(1, 2 * F).astype(bf16)
    ident = np.eye(128, dtype=bf16)
    ones1 = np.ones((1, 128), dtype=bf16)

    in_maps = []
    for m in range(NCORES):
        gs = range(m * GPC, (m + 1) * GPC)
        haug = np.zeros((GPC, 128, NB, FP), dtype=bf16)
        ht = np.zeros((GPC, 128, n), dtype=bf16)
        sidx = np.zeros((GPC, 128, NB, Spad), dtype=np.int16)
        scnt = np.zeros((GPC, 128, NB, Spad), dtype=bf16)
        for i, g in enumerate(gs):
            hg = hb[g * n:(g + 1) * n]                     # [1024, 128] bf16
            # natural layout [p, b, f] for node u = b*128+p
            haug[i, :, :, :F] = hg.reshape(NB, 128, F).transpose(1, 0, 2)
            haug[i, :, :, F] = 1.0
            ht[i] = hg.T                                   # [F, 1024]
            sidx[i], scnt[i] = packs[g]
        in_maps.append({
            "haug": haug, "ht": ht, "sidx": sidx, "scnt": scnt,
            "w1": w1, "w2": w2, "brow": brow, "ident": ident, "ones1": ones1,
        })
    return in_maps, Spad


# ---------------------------------------------------------------- device IR

def _emit(ctx: ExitStack, tc: tile.TileContext, P, Spad):
    nc = tc.nc
    ts = bass.ts

    consts = ctx.enter_context(tc.tile_pool(name="consts", bufs=1))
    pin = ctx.enter_context(tc.tile_pool(name="pin", bufs=2))
    pA = ctx.enter_context(tc.tile_pool(name="pA", bufs=2))
    pmid = ctx.enter_context(tc.tile_pool(name="pmid", bufs=2))
    pout = ctx.enter_context(tc.tile_pool(name="pout", bufs=2))
    ps_c = ctx.enter_context(tc.tile_pool(name="ps_c", bufs=2, space="PSUM"))
    ps_ct = ctx.enter_context(tc.tile_pool(name="ps_ct", bufs=1, space="PSUM"))
    ps_z = ctx.enter_context(tc.tile_pool(name="ps_z", bufs=2, space="PSUM"))
    ps_at = ctx.enter_context(tc.tile_pool(name="ps_at", bufs=1, space="PSUM"))
    ps_as = ctx.enter_context(tc.tile_pool(name="ps_as", bufs=2, space="PSUM"))

    # constants, loaded once
    w1 = consts.tile([128, 2 * F], dt.bfloat16)
    w2 = consts.tile([128, 2 * F], dt.bfloat16)
    brow = consts.tile([1, 2 * F], dt.bfloat16)
    ident = consts.tile([128, 128], dt.bfloat16)
    ones1 = consts.tile([1, 128], dt.bfloat16)
    nc.sync.dma_start(out=w1[:], in_=P["w1"].ap())
    nc.sync.dma_start(out=w2[:], in_=P["w2"].ap())
    nc.sync.dma_start(out=brow[:], in_=P["brow"].ap())
    nc.sync.dma_start(out=ident[:], in_=P["ident"].ap())
    nc.sync.dma_start(out=ones1[:], in_=P["ones1"].ap())

    for g in range(GPC):
        # ---- loads
        ha = pin.tile([128, NB, FP], dt.bfloat16, tag="ha")
        ht = pin.tile([128, n], dt.bfloat16, tag="ht")
        six = pin.tile([128, NB, Spad], dt.int16, tag="six")
        sct = pin.tile([128, NB, Spad], dt.bfloat16, tag="sct")
        nc.sync.dma_start(out=ha[:], in_=P["haug"].ap()[g])
        nc.sync.dma_start(out=ht[:], in_=P["ht"].ap()[g])
        nc.sync.dma_start(out=six[:], in_=P["sidx"].ap()[g])
        nc.sync.dma_start(out=sct[:], in_=P["scnt"].ap()[g])

        # ---- A build: A[u_lo, u_hi, v] = count(u -> v)
        A = pA.tile([128, NB, n], dt.bfloat16, tag="A")
        for b in range(NB):
            nc.gpsimd.local_scatter(
                A[:, b, :], sct[:, b, :], six[:, b, :],
                channels=128, num_elems=n, num_idxs=Spad,
            )

        # ---- c_aug[v, :] = sum_u A[u, v] * h_aug[u, :]; col F = in-degree
        c = pmid.tile([128, NB, F], dt.bfloat16, tag="c")
        cT = pmid.tile([128, NB, 128], dt.bfloat16, tag="cT")
        for vt in range(NB):
            cps = ps_c.tile([128, F + 1], dt.float32, tag="cps")
            for j in range(NB):
                nc.tensor.matmul(
                    cps[:], A[:, j, ts(vt, 128)], ha[:, j, 0:F + 1],
                    start=(j == 0), stop=(j == NB - 1),
                )
            rdeg = pmid.tile([128, 1], dt.float32, tag="rdeg")
            nc.vector.tensor_scalar_max(rdeg[:], cps[:, F:F + 1], 1.0)
            nc.vector.reciprocal(rdeg[:], rdeg[:])
            # c (bf16, natural) = c_sum * (1/deg)
            nc.scalar.activation(c[:, vt, :], cps[:, 0:F], AF.Copy, scale=rdeg[:])
        # c^T tiles for the linear layers
        ctps = ps_ct.tile([128, NB, 128], dt.bfloat16, tag="ctps")
        for vt in range(NB):
            nc.tensor.transpose(ctps[:, vt, :], c[:, vt, :], ident[:])
        nc.vector.tensor_copy(cT[:], ctps[:])

        # ---- z = [h, c] @ [W_feat | W_pool] + b ; per 128-node tile
        FA = pmid.tile([128, NB, 2 * F], dt.bfloat16, tag="FA")   # feat | AS
        S = pmid.tile([128, NB, F], dt.bfloat16, tag="S")
        for it in range(NB):
            zps = ps_z.tile([128, 2 * F], dt.float32, tag="zps")
            nc.tensor.matmul(zps[:], ht[:, ts(it, 128)], w1[:], start=True, stop=False)
            nc.tensor.matmul(zps[:], cT[:, it, :], w2[:], start=False, stop=False)
            nc.tensor.matmul(zps[:], ones1[:], brow[:], start=False, stop=True)

            sq = pmid.tile([128, 2 * F], dt.float32, tag="sq")
            n2 = pmid.tile([128, 2], dt.float32, tag="n2")
            nc.scalar.activation(sq[:, 0:F], zps[:, 0:F], AF.Square,
                                 accum_out=n2[:, 0:1])
            nc.scalar.activation(sq[:, F:], zps[:, F:], AF.Square,
                                 accum_out=n2[:, 1:2])
            rr = pmid.tile([128, 2], dt.float32, tag="rr")
            nc.scalar.activation(rr[:], n2[:], AF.Sqrt)
            nc.vector.tensor_scalar_max(rr[:], rr[:], EPS)
            nc.vector.reciprocal(rr[:], rr[:])
            # feat tile (bf16) = relu(z_feat / ||z_feat||)
            nc.scalar.activation(FA[:, it, 0:F], zps[:, 0:F], AF.Relu,
                                 scale=rr[:, 0:1])
            yp = pmid.tile([128, F], dt.float32, tag="yp")
            nc.scalar.activation(yp[:], zps[:, F:], AF.Relu, scale=rr[:, 1:2])
            # softmax over K
            negm = pmid.tile([128, 1], dt.float32, tag="negm")
            nc.vector.reduce_max(negm[:], yp[:], axis=mybir.AxisListType.X,
                                 negate=True)
            ey = pmid.tile([128, F], dt.float32, tag="ey")
            se = pmid.tile([128, 1], dt.float32, tag="se")
            nc.scalar.activation(ey[:], yp[:], AF.Exp, bias=negm[:],
                                 accum_out=se[:])
            nc.vector.reciprocal(se[:], se[:])
            nc.scalar.activation(S[:, it, :], ey[:], AF.Copy, scale=se[:])

        # ---- A^T via PE transposes
        AT = pA.tile([128, NB, n], dt.bfloat16, tag="AT")
        for jj in range(NB):
            atps = ps_at.tile([128, n], dt.bfloat16, tag="atps")
            for j in range(NB):
                nc.tensor.transpose(atps[:, ts(j, 128)], A[:, j, ts(jj, 128)],
                                    ident[:])
            nc.vector.tensor_copy(AT[:, jj, :], atps[:])

        # ---- AS[u, :] = sum_v A[u, v] S[v, :]
        for ut in range(NB):
            asps = ps_as.tile([128, F], dt.float32, tag="asps")
            for jj in range(NB):
                nc.tensor.matmul(
                    asps[:], AT[:, jj, ts(ut, 128)], S[:, jj, :],
                    start=(jj == 0), stop=(jj == NB - 1),
                )
            nc.scalar.activation(FA[:, ut, F:], asps[:], AF.Copy)

        # ---- h_new | blocks = S^T @ [feat | AS]
        fin = ps_z.tile([128, 2 * F], dt.float32, tag="zps")
        for j in range(NB):
            nc.tensor.matmul(fin[:], S[:, j, :], FA[:, j, :],
                             start=(j == 0), stop=(j == NB - 1))
        ot = pout.tile([128, 2 * F], dt.float32, tag="ot")
        nc.scalar.activation(ot[:], fin[:], AF.Copy)
        nc.sync.dma_start(out=P["out_hb"].ap()[g], in_=ot[:])


def _build(Spad):
    if Spad in _compiled:
        return _compiled[Spad]
    nc = bacc.Bacc("TRN2", target_bir_lowering=False, debug=False)
    P = {
        "haug": nc.declare_dram_parameter("haug", [GPC, 128, NB, FP], dt.bfloat16, isOutput=False),
        "ht": nc.declare_dram_parameter("ht", [GPC, 128, n], dt.bfloat16, isOutput=False),
        "sidx": nc.declare_dram_parameter("sidx", [GPC, 128, NB, Spad], dt.int16, isOutput=False),
        "scnt": nc.declare_dram_parameter("scnt", [GPC, 128, NB, Spad], dt.bfloat16, isOutput=False),
        "w1": nc.declare_dram_parameter("w1", [128, 2 * F], dt.bfloat16, isOutput=False),
        "w2": nc.declare_dram_parameter("w2", [128, 2 * F], dt.bfloat16, isOutput=False),
        "brow": nc.declare_dram_parameter("brow", [1, 2 * F], dt.bfloat16, isOutput=False),
        "ident": nc.declare_dram_parameter("ident", [128, 128], dt.bfloat16, isOutput=False),
        "ones1": nc.declare_dram_parameter("ones1", [1, 128], dt.bfloat16, isOutput=False),
        "out_hb": nc.declare_dram_parameter("out_hb", [GPC, 128, 2 * F], dt.float32, isOutput=True),
    }
    with tile.TileContext(nc) as tc, ExitStack() as ctx:
        _emit(ctx, tc, P, Spad)
    nc.compile()
    _compiled[Spad] = nc
    return nc


# ---------------------------------------------------------------- entry

def kernel(h, src_local, dst_local, W_feat, b_feat, W_pool, b_pool,
           _trace=False, _tmpdir=None):
    in_maps, Spad = _host_prep(h, src_local, dst_local,
                               W_feat, b_feat, W_pool, b_pool)
    nc = _build(Spad)
    res = run_bass_kernel_spmd(nc, in_maps, core_ids=list(range(NCORES)),
                               trace=_trace, tmpdir=_tmpdir)
    kernel.last_results = res

    h_new = np.zeros((B * K, F), dtype=np.float32)
    adj_new = np.zeros((B * K, B * K), dtype=np.float32)
    for m in range(NCORES):
        out = np.asarray(res.results[m]["out_hb"])     # [GPC, 128, 256] f32
        for i in range(GPC):
            g = m * GPC + i
            h_new[g * K:(g + 1) * K] = out[i, :, :F]
            adj_new[g * K:(g + 1) * K, g * K:(g + 1) * K] = out[i, :, F:]
    return adj_new, h_new


# revision 7
# speedup vs baseline: 1.3041x; 1.3041x over previous
"""DiffPool batched-graph layer on 8 TRN2 NeuronCores (Bass/Tile).

Strategy: shard the B=64 graphs across 8 cores (8 graphs/core). All segment
ops become dense per-graph matmuls against the graph's adjacency-count matrix
A [n, n], which is built on-chip in SBUF by GPSIMD local_scatter from
host-packed (dedup'd, per-src-node padded) edge lists. A^T comes from PE
transposes. The mean-aggregator degree falls out of the same matmul via an
appended ones-column on h. Final h_new/blocks are per-graph [128, 256] tiles;
the host assembles the block-diagonal adj_new.
"""
from contextlib import ExitStack

import numpy as np
import ml_dtypes

import concourse.bass as bass
import concourse.bacc as bacc
import concourse.mybir as mybir
from concourse import tile
from concourse.bass_utils import run_bass_kernel_spmd

bf16 = ml_dtypes.bfloat16
AF = mybir.ActivationFunctionType
dt = mybir.dt

# Problem constants (hardcoded per harness contract)
B, n, F, K, E = 64, 1024, 128, 128, 16384
NCORES = 8
GPC = B // NCORES       # graphs per core
NB = n // 128           # 128-row blocks per graph
EPS = 1e-12
FP = F + 2              # h_aug free: F features + ones col + pad

_compiled = {}          # Spad -> compiled Bacc


# ---------------------------------------------------------------- host prep

def _pack_graph_edges(src, dst, Spad):
    """One graph's edges -> (idx int16 [128, NB, Spad], cnt bf16 same shape).

    Dedup (src,dst) with counts; node u = b*128+p gets its (dst, count) list in
    partition p, block b, padded with -1.
    """
    key = src.astype(np.int64) * n + dst.astype(np.int64)
    uniq, counts = np.unique(key, return_counts=True)
    us = (uniq // n).astype(np.int64)
    vs = (uniq % n).astype(np.int64)
    # slot j within each run of equal us (uniq is sorted so us runs are contiguous)
    starts = np.r_[0, np.flatnonzero(np.diff(us)) + 1]
    run_len = np.diff(np.r_[starts, len(us)])
    j = np.arange(len(us)) - np.repeat(starts, run_len)
    smax = int(j.max()) + 1 if len(j) else 1
    if smax > Spad:
        raise ValueError(f"Spad {Spad} too small, need {smax}")
    idx = np.full((128, NB, Spad), -1, dtype=np.int16)
    cnt = np.zeros((128, NB, Spad), dtype=bf16)
    idx[us % 128, us // 128, j] = vs.astype(np.int16)
    cnt[us % 128, us // 128, j] = counts.astype(bf16)
    return idx, cnt


def _host_prep(h, src_local, dst_local, W_feat, b_feat, W_pool, b_pool):
    h = np.asarray(h, dtype=np.float32)
    src_local = np.asarray(src_local)
    dst_local = np.asarray(dst_local)

    # global Spad (compile-time constant), rounded up to multiple of 4
    deg_max = 0
    packs = []
    # first pass to find Spad cheaply: count max multiplicity-run per src node
    for g in range(B):
        key = src_local[g].astype(np.int64) * n + dst_local[g].astype(np.int64)
        uniq = np.unique(key)
        us = uniq // n
        _, c = np.unique(us, return_counts=True)
        deg_max = max(deg_max, int(c.max()))
    Spad = (deg_max + 3) // 4 * 4

    for g in range(B):
        packs.append(_pack_graph_edges(src_local[g], dst_local[g], Spad))

    hb = h.astype(bf16)                       # [N, F] bf16
    w1 = np.concatenate([W_feat[:F], W_pool[:F]], axis=1).astype(bf16)   # [128, 256]
    w2 = np.concatenate([W_feat[F:], W_pool[F:]], axis=1).astype(bf16)   # [128, 256]
    brow = np.concatenate([b_feat, b_pool]).reshape(1, 2 * F).astype(bf16)
    ident = np.eye(128, dtype=bf16)
    ones1 = np.ones((1, 128), dtype=bf16)

    in_maps = []
    for m in range(NCORES):
        gs = range(m * GPC, (m + 1) * GPC)
        haug = np.zeros((GPC, 128, NB, FP), dtype=bf16)
        ht = np.zeros((GPC, 128, n), dtype=bf16)
        sidx = np.zeros((GPC, 128, NB, Spad), dtype=np.int16)
        scnt = np.zeros((GPC, 128, NB, Spad), dtype=bf16)
        for i, g in enumerate(gs):
            hg = hb[g * n:(g + 1) * n]                     # [1024, 128] bf16
            # natural layout [p, b, f] for node u = b*128+p
            haug[i, :, :, :F] = hg.reshape(NB, 128, F).transpose(1, 0, 2)
            haug[i, :, :, F] = 1.0
            ht[i] = hg.T                                   # [F, 1024]
            sidx[i], scnt[i] = packs[g]
        in_maps.append({
            "haug": haug, "ht": ht, "sidx": sidx, "scnt": scnt,
            "w1": w1, "w2": w2, "brow": brow, "ident": ident, "ones1": ones1,
        })
    return in_maps, Spad


# ---------------------------------------------------------------- device IR

def _emit(ctx: ExitStack, tc: tile.TileContext, P, Spad, has_bias):
    nc = tc.nc
    ts = bass.ts

    consts = ctx.enter_context(tc.tile_pool(name="consts", bufs=1))
    pin = ctx.enter_context(tc.tile_pool(name="pin", bufs=2))
    pA = ctx.enter_context(tc.tile_pool(name="pA", bufs=2))
    pmid = ctx.enter_context(tc.tile_pool(name="pmid", bufs=2))
    pout = ctx.enter_context(tc.tile_pool(name="pout", bufs=2))
    ps_c = ctx.enter_context(tc.tile_pool(name="ps_c", bufs=2, space="PSUM"))
    ps_ct = ctx.enter_context(tc.tile_pool(name="ps_ct", bufs=1, space="PSUM"))
    ps_z = ctx.enter_context(tc.tile_pool(name="ps_z", bufs=2, space="PSUM"))
    ps_t = ctx.enter_context(tc.tile_pool(name="ps_t", bufs=2, space="PSUM"))

    # constants, loaded once
    w1 = consts.tile([128, 2 * F], dt.bfloat16)
    w2 = consts.tile([128, 2 * F], dt.bfloat16)
    brow = consts.tile([1, 2 * F], dt.bfloat16)
    ident = consts.tile([128, 128], dt.bfloat16)
    ones1 = consts.tile([1, 128], dt.bfloat16)
    magic = consts.tile([128, 2 * NB], dt.int32)
    nc.sync.dma_start(out=w1[:], in_=P["w1"].ap())
    nc.sync.dma_start(out=w2[:], in_=P["w2"].ap())
    nc.sync.dma_start(out=brow[:], in_=P["brow"].ap())
    nc.sync.dma_start(out=ident[:], in_=P["ident"].ap())
    nc.sync.dma_start(out=ones1[:], in_=P["ones1"].ap())
    nc.vector.memset(magic[:], 0x5F3759DF)

    for g in range(GPC):
        # ---- loads
        ha = pin.tile([128, NB, FP], dt.bfloat16, tag="ha")
        ht = pin.tile([128, n], dt.bfloat16, tag="ht")
        six = pin.tile([128, NB, Spad], dt.int16, tag="six")
        sct = pin.tile([128, NB, Spad], dt.bfloat16, tag="sct")
        nc.sync.dma_start(out=ha[:], in_=P["haug"].ap()[g])
        nc.sync.dma_start(out=ht[:], in_=P["ht"].ap()[g])
        nc.sync.dma_start(out=six[:], in_=P["sidx"].ap()[g])
        nc.sync.dma_start(out=sct[:], in_=P["scnt"].ap()[g])

        # ---- A build: A[u_lo, u_hi, v] = count(u -> v)
        A = pA.tile([128, NB, n], dt.bfloat16, tag="A")
        for b in range(NB):
            nc.gpsimd.local_scatter(
                A[:, b, :], sct[:, b, :], six[:, b, :],
                channels=128, num_elems=n, num_idxs=Spad,
            )

        # ---- c_aug[v, :] = sum_u A[u, v] * h_aug[u, :]; col F = in-degree
        c = pmid.tile([128, NB, F], dt.bfloat16, tag="c")
        cT = pmid.tile([128, NB, 128], dt.bfloat16, tag="cT")
        for vt in range(NB):
            cps = ps_c.tile([128, F + 1], dt.float32, tag="cps")
            for j in range(NB):
                nc.tensor.matmul(
                    cps[:], A[:, j, ts(vt, 128)], ha[:, j, 0:F + 1],
                    start=(j == 0), stop=(j == NB - 1),
                )
            rdeg = pmid.tile([128, 1], dt.float32, tag="rdeg")
            nc.vector.tensor_scalar_max(rdeg[:], cps[:, F:F + 1], 1.0)
            nc.vector.reciprocal(rdeg[:], rdeg[:])
            # c (bf16, natural) = c_sum * (1/deg)   [DVE: per-partition scalar]
            nc.vector.tensor_scalar_mul(c[:, vt, :], cps[:, 0:F], rdeg[:])
        # c^T tiles for the linear layers
        ctps = ps_ct.tile([128, NB, 128], dt.bfloat16, tag="ctps")
        for vt in range(NB):
            nc.tensor.transpose(ctps[:, vt, :], c[:, vt, :], ident[:])
        nc.vector.tensor_copy(cT[:], ctps[:])

        # ---- z = [h, c] @ [W_feat | W_pool] (+ b); copy out; sum of squares
        zall = pmid.tile([128, NB, 2 * F], dt.float32, tag="zall")
        n2 = pmid.tile([128, 2 * NB], dt.float32, tag="n2")
        sq = pmid.tile([128, 2 * F], dt.float32, tag="sq")
        for it in range(NB):
            zps = ps_z.tile([128, 2 * F], dt.float32, tag="zps")
            nc.tensor.matmul(zps[:], ht[:, ts(it, 128)], w1[:], start=True,
                             stop=False)
            nc.tensor.matmul(zps[:], cT[:, it, :], w2[:], start=False,
                             stop=not has_bias)
            if has_bias:
                nc.tensor.matmul(zps[:], ones1[:], brow[:], start=False,
                                 stop=True)
            nc.scalar.activation(zall[:, it, :], zps[:], AF.Copy)
            import os as _os
            if False:
                nc.vector.tensor_tensor_reduce(
                    sq[:, 0:F], zall[:, it, 0:F], zall[:, it, 0:F], 1.0, 0.0,
                    mybir.AluOpType.mult, mybir.AluOpType.add,
                    accum_out=n2[:, 2 * it:2 * it + 1])
                nc.vector.tensor_tensor_reduce(
                    sq[:, F:], zall[:, it, F:], zall[:, it, F:], 1.0, 0.0,
                    mybir.AluOpType.mult, mybir.AluOpType.add,
                    accum_out=n2[:, 2 * it + 1:2 * it + 2])
            else:
                nc.scalar.activation(sq[:, 0:F], zall[:, it, 0:F], AF.Square,
                                     accum_out=n2[:, 2 * it:2 * it + 1])
                nc.scalar.activation(sq[:, F:], zall[:, it, F:], AF.Square,
                                     accum_out=n2[:, 2 * it + 1:2 * it + 2])

        # ---- rr = rsqrt(max(n2, eps^2)) via magic-seed + 3 Newton steps (DVE)
        rr = pmid.tile([128, 2 * NB], dt.float32, tag="rr")
        t0 = pmid.tile([128, 2 * NB], dt.float32, tag="t0")
        import os as _os
        if True:
            nc.vector.tensor_scalar_max(rr[:], n2[:], EPS * EPS)
            rri = rr[:].bitcast(dt.int32)
            t0i = t0[:].bitcast(dt.int32)
            nc.vector.tensor_scalar(t0i, rri, 1, None,
                                    mybir.AluOpType.logical_shift_right)
            nc.vector.tensor_tensor(t0i, magic[:], t0i, mybir.AluOpType.subtract)
            # y = y * (1.5 - 0.5 * x * y^2), x = clamped n2 (in rr)
            for _ in range(3):
                y2 = pmid.tile([128, 2 * NB], dt.float32, tag="y2")
                nc.vector.tensor_tensor(y2[:], t0[:], t0[:], mybir.AluOpType.mult)
                nc.vector.tensor_tensor(y2[:], y2[:], rr[:], mybir.AluOpType.mult)
                nc.vector.tensor_scalar(y2[:], y2[:], -0.5, 1.5,
                                        mybir.AluOpType.mult, mybir.AluOpType.add)
                nc.vector.tensor_tensor(t0[:], t0[:], y2[:], mybir.AluOpType.mult)
        else:
            nc.scalar.activation(rr[:], n2[:], AF.Sqrt)
            nc.vector.tensor_scalar_max(rr[:], rr[:], EPS)
            nc.vector.reciprocal(t0[:], rr[:])

        # ---- feat / softmax(assign) per tile; Act stays in one table set
        FA = pmid.tile([128, NB, F], dt.bfloat16, tag="FA")       # feat
        S = pmid.tile([128, NB, F], dt.bfloat16, tag="S")
        for it in range(NB):
            nc.scalar.activation(FA[:, it, :], zall[:, it, 0:F], AF.Relu,
                                 scale=t0[:, 2 * it:2 * it + 1])
            yp = pmid.tile([128, F], dt.float32, tag="yp")
            nc.scalar.activation(yp[:], zall[:, it, F:], AF.Relu,
                                 scale=t0[:, 2 * it + 1:2 * it + 2])
            # softmax over K: values in [0, 1] so no max-shift needed
            ey = pmid.tile([128, F], dt.float32, tag="ey")
            se = pmid.tile([128, 1], dt.float32, tag="se")
            nc.scalar.activation(ey[:], yp[:], AF.Exp, accum_out=se[:])
            nc.vector.reciprocal(se[:], se[:])
            nc.vector.tensor_scalar_mul(S[:, it, :], ey[:], se[:])

        # ---- T = A^T S (same orientation as c); blocks = T^T S
        T = pmid.tile([128, NB, F], dt.bfloat16, tag="T")
        for vt in range(NB):
            tps = ps_t.tile([128, F], dt.float32, tag="tps")
            for j in range(NB):
                nc.tensor.matmul(
                    tps[:], A[:, j, ts(vt, 128)], S[:, j, :],
                    start=(j == 0), stop=(j == NB - 1),
                )
            nc.scalar.activation(T[:, vt, :], tps[:], AF.Copy)

        # ---- h_new = S^T feat ; blocks = T^T S
        hps = ps_t.tile([128, F], dt.float32, tag="tps")
        for j in range(NB):
            nc.tensor.matmul(hps[:], S[:, j, :], FA[:, j, :],
                             start=(j == 0), stop=(j == NB - 1))
        ot = pout.tile([128, 2 * F], dt.float32, tag="ot")
        nc.scalar.activation(ot[:, 0:F], hps[:], AF.Copy)
        bps = ps_t.tile([128, F], dt.float32, tag="tps")
        for j in range(NB):
            nc.tensor.matmul(bps[:], T[:, j, :], S[:, j, :],
                             start=(j == 0), stop=(j == NB - 1))
        nc.scalar.activation(ot[:, F:], bps[:], AF.Copy)
        nc.sync.dma_start(out=P["out_hb"].ap()[g], in_=ot[:])


def _build(Spad, has_bias):
    key = (Spad, has_bias)
    if key in _compiled:
        return _compiled[key]
    nc = bacc.Bacc("TRN2", target_bir_lowering=False, debug=False)
    P = {
        "haug": nc.declare_dram_parameter("haug", [GPC, 128, NB, FP], dt.bfloat16, isOutput=False),
        "ht": nc.declare_dram_parameter("ht", [GPC, 128, n], dt.bfloat16, isOutput=False),
        "sidx": nc.declare_dram_parameter("sidx", [GPC, 128, NB, Spad], dt.int16, isOutput=False),
        "scnt": nc.declare_dram_parameter("scnt", [GPC, 128, NB, Spad], dt.bfloat16, isOutput=False),
        "w1": nc.declare_dram_parameter("w1", [128, 2 * F], dt.bfloat16, isOutput=False),
        "w2": nc.declare_dram_parameter("w2", [128, 2 * F], dt.bfloat16, isOutput=False),
        "brow": nc.declare_dram_parameter("brow", [1, 2 * F], dt.bfloat16, isOutput=False),
        "ident": nc.declare_dram_parameter("ident", [128, 128], dt.bfloat16, isOutput=False),
        "ones1": nc.declare_dram_parameter("ones1", [1, 128], dt.bfloat16, isOutput=False),
        "out_hb": nc.declare_dram_parameter("out_hb", [GPC, 128, 2 * F], dt.float32, isOutput=True),
    }
    with tile.TileContext(nc) as tc, ExitStack() as ctx:
        _emit(ctx, tc, P, Spad, has_bias)
    nc.compile()
    _compiled[key] = nc
    return nc


# ---------------------------------------------------------------- entry

def kernel(h, src_local, dst_local, W_feat, b_feat, W_pool, b_pool,
           _trace=False, _tmpdir=None):
    in_maps, Spad = _host_prep(h, src_local, dst_local,
                               W_feat, b_feat, W_pool, b_pool)
    has_bias = bool(np.any(np.asarray(b_feat)) or np.any(np.asarray(b_pool)))
    nc = _build(Spad, has_bias)
    res = run_bass_kernel_spmd(nc, in_maps, core_ids=list(range(NCORES)),
                               trace=_trace, tmpdir=_tmpdir)
    kernel.last_results = res

    h_new = np.zeros((B * K, F), dtype=np.float32)
    adj_new = np.zeros((B * K, B * K), dtype=np.float32)
    for m in range(NCORES):
        out = np.asarray(res.results[m]["out_hb"])     # [GPC, 128, 256] f32
        for i in range(GPC):
            g = m * GPC + i
            h_new[g * K:(g + 1) * K] = out[i, :, :F]
            adj_new[g * K:(g + 1) * K, g * K:(g + 1) * K] = out[i, :, F:]
    return adj_new, h_new


# revision 9
# speedup vs baseline: 1.4975x; 1.1483x over previous
"""DiffPool batched-graph layer on 8 TRN2 NeuronCores (Bass/Tile).

Strategy: shard the B=64 graphs across 8 cores (8 graphs/core). All segment
ops become dense per-graph matmuls against the graph's adjacency-count matrix
A [n, n], which is built on-chip in SBUF by GPSIMD local_scatter from
host-packed (dedup'd, per-src-node padded) edge lists. A^T comes from PE
transposes. The mean-aggregator degree falls out of the same matmul via an
appended ones-column on h. Final h_new/blocks are per-graph [128, 256] tiles;
the host assembles the block-diagonal adj_new.
"""
from contextlib import ExitStack

import numpy as np
import ml_dtypes

import concourse.bass as bass
import concourse.bacc as bacc
import concourse.mybir as mybir
from concourse import tile
from concourse.bass_utils import run_bass_kernel_spmd

bf16 = ml_dtypes.bfloat16
AF = mybir.ActivationFunctionType
dt = mybir.dt

# Problem constants (hardcoded per harness contract)
B, n, F, K, E = 64, 1024, 128, 128, 16384
NCORES = 8
GPC = B // NCORES       # graphs per core
NB = n // 128           # 128-row blocks per graph
EPS = 1e-12
FP = F + 2              # h_aug free: F features + ones col + pad

_compiled = {}          # Spad -> compiled Bacc


# ---------------------------------------------------------------- host prep

def _pack_graph_edges(src, dst, Spad):
    """One graph's edges -> (idx int16 [128, NB, Spad], cnt bf16 same shape).

    Dedup (src,dst) with counts; node u = b*128+p gets its (dst, count) list in
    partition p, block b, padded with -1.
    """
    key = src.astype(np.int64) * n + dst.astype(np.int64)
    uniq, counts = np.unique(key, return_counts=True)
    us = (uniq // n).astype(np.int64)
    vs = (uniq % n).astype(np.int64)
    # slot j within each run of equal us (uniq is sorted so us runs are contiguous)
    starts = np.r_[0, np.flatnonzero(np.diff(us)) + 1]
    run_len = np.diff(np.r_[starts, len(us)])
    j = np.arange(len(us)) - np.repeat(starts, run_len)
    smax = int(j.max()) + 1 if len(j) else 1
    if smax > Spad:
        raise ValueError(f"Spad {Spad} too small, need {smax}")
    idx = np.full((128, NB, Spad), -1, dtype=np.int16)
    cnt = np.zeros((128, NB, Spad), dtype=bf16)
    idx[us % 128, us // 128, j] = vs.astype(np.int16)
    cnt[us % 128, us // 128, j] = counts.astype(bf16)
    return idx, cnt


def _host_prep(h, src_local, dst_local, W_feat, b_feat, W_pool, b_pool):
    h = np.asarray(h, dtype=np.float32)
    src_local = np.asarray(src_local)
    dst_local = np.asarray(dst_local)

    # global Spad (compile-time constant), rounded up to multiple of 4
    deg_max = 0
    packs = []
    # first pass to find Spad cheaply: count max multiplicity-run per src node
    for g in range(B):
        key = src_local[g].astype(np.int64) * n + dst_local[g].astype(np.int64)
        uniq = np.unique(key)
        us = uniq // n
        _, c = np.unique(us, return_counts=True)
        deg_max = max(deg_max, int(c.max()))
    Spad = (deg_max + 3) // 4 * 4

    for g in range(B):
        packs.append(_pack_graph_edges(src_local[g], dst_local[g], Spad))

    hb = h.astype(bf16)                       # [N, F] bf16
    w1 = np.concatenate([W_feat[:F], W_pool[:F]], axis=1).astype(bf16)   # [128, 256]
    w2 = np.concatenate([W_feat[F:], W_pool[F:]], axis=1).astype(bf16)   # [128, 256]
    brow = np.concatenate([b_feat, b_pool]).reshape(1, 2 * F).astype(bf16)
    ident = np.eye(128, dtype=bf16)
    ones1 = np.ones((1, 128), dtype=bf16)

    in_maps = []
    for m in range(NCORES):
        gs = range(m * GPC, (m + 1) * GPC)
        haug = np.zeros((GPC, 128, NB, FP), dtype=bf16)
        ht = np.zeros((GPC, 128, n), dtype=bf16)
        sidx = np.zeros((GPC, 128, NB, Spad), dtype=np.int16)
        scnt = np.zeros((GPC, 128, NB, Spad), dtype=bf16)
        for i, g in enumerate(gs):
            hg = hb[g * n:(g + 1) * n]                     # [1024, 128] bf16
            # natural layout [p, b, f] for node u = b*128+p
            haug[i, :, :, :F] = hg.reshape(NB, 128, F).transpose(1, 0, 2)
            haug[i, :, :, F] = 1.0
            ht[i] = hg.T                                   # [F, 1024]
            sidx[i], scnt[i] = packs[g]
        in_maps.append({
            "haug": haug, "ht": ht, "sidx": sidx, "scnt": scnt,
            "w1": w1, "w2": w2, "brow": brow, "ident": ident, "ones1": ones1,
        })
    return in_maps, Spad


# ---------------------------------------------------------------- device IR

def _emit(ctx: ExitStack, tc: tile.TileContext, P, Spad, has_bias):
    nc = tc.nc
    ts = bass.ts

    consts = ctx.enter_context(tc.tile_pool(name="consts", bufs=1))
    pin = ctx.enter_context(tc.tile_pool(name="pin", bufs=2))
    pA = ctx.enter_context(tc.tile_pool(name="pA", bufs=2))
    pmid = ctx.enter_context(tc.tile_pool(name="pmid", bufs=2))
    pout = ctx.enter_context(tc.tile_pool(name="pout", bufs=2))
    ps_c = ctx.enter_context(tc.tile_pool(name="ps_c", bufs=2, space="PSUM"))
    ps_ct = ctx.enter_context(tc.tile_pool(name="ps_ct", bufs=1, space="PSUM"))
    ps_z = ctx.enter_context(tc.tile_pool(name="ps_z", bufs=2, space="PSUM"))
    ps_t = ctx.enter_context(tc.tile_pool(name="ps_t", bufs=1, space="PSUM"))

    # constants, loaded once
    w1 = consts.tile([128, 2 * F], dt.bfloat16)
    w2 = consts.tile([128, 2 * F], dt.bfloat16)
    brow = consts.tile([1, 2 * F], dt.bfloat16)
    ident = consts.tile([128, 128], dt.bfloat16)
    ones1 = consts.tile([1, 128], dt.bfloat16)
    magic = consts.tile([128, NB, 2], dt.int32)
    nc.sync.dma_start(out=w1[:], in_=P["w1"].ap())
    nc.sync.dma_start(out=w2[:], in_=P["w2"].ap())
    nc.sync.dma_start(out=brow[:], in_=P["brow"].ap())
    nc.sync.dma_start(out=ident[:], in_=P["ident"].ap())
    nc.sync.dma_start(out=ones1[:], in_=P["ones1"].ap())
    nc.vector.memset(magic[:], 0x5F3759DF)

    for g in range(GPC):
        # ---- loads
        ha = pin.tile([128, NB, FP], dt.bfloat16, tag="ha")
        ht = pin.tile([128, n], dt.bfloat16, tag="ht")
        six = pin.tile([128, NB, Spad], dt.int16, tag="six")
        sct = pin.tile([128, NB, Spad], dt.bfloat16, tag="sct")
        nc.sync.dma_start(out=ha[:], in_=P["haug"].ap()[g])
        nc.sync.dma_start(out=ht[:], in_=P["ht"].ap()[g])
        nc.sync.dma_start(out=six[:], in_=P["sidx"].ap()[g])
        nc.sync.dma_start(out=sct[:], in_=P["scnt"].ap()[g])

        # ---- A build: A[u_lo, u_hi, v] = count(u -> v)
        A = pA.tile([128, NB, n], dt.bfloat16, tag="A")
        for b in range(NB):
            nc.gpsimd.local_scatter(
                A[:, b, :], sct[:, b, :], six[:, b, :],
                channels=128, num_elems=n, num_idxs=Spad,
            )

        # ---- c_aug[v, :] = sum_u A[u, v] * h_aug[u, :]; col F = in-degree
        c = pmid.tile([128, NB, F], dt.bfloat16, tag="c")
        cT = pmid.tile([128, NB, 128], dt.bfloat16, tag="cT")
        for vt in range(NB):
            cps = ps_c.tile([128, F + 1], dt.float32, tag="cps")
            for j in range(NB):
                nc.tensor.matmul(
                    cps[:], A[:, j, ts(vt, 128)], ha[:, j, 0:F + 1],
                    start=(j == 0), stop=(j == NB - 1),
                )
            rdeg = pmid.tile([128, 1], dt.float32, tag="rdeg")
            nc.vector.tensor_scalar_max(rdeg[:], cps[:, F:F + 1], 1.0)
            nc.vector.reciprocal(rdeg[:], rdeg[:])
            # c (bf16, natural) = c_sum * (1/deg)   [DVE: per-partition scalar]
            nc.vector.tensor_scalar_mul(c[:, vt, :], cps[:, 0:F], rdeg[:])
        # c^T tiles for the linear layers
        ctps = ps_ct.tile([128, NB, 128], dt.bfloat16, tag="ctps")
        for vt in range(NB):
            nc.tensor.transpose(ctps[:, vt, :], c[:, vt, :], ident[:])
        nc.vector.tensor_copy(cT[:], ctps[:])

        # ---- z = [h, c] @ [W_feat | W_pool] (+ b) -> zall (bf16)
        zall = pmid.tile([128, NB, 2, F], dt.bfloat16, tag="zall")
        for it in range(NB):
            zps = ps_z.tile([128, 2 * F], dt.float32, tag="zps")
            nc.tensor.matmul(zps[:], ht[:, ts(it, 128)], w1[:], start=True,
                             stop=False)
            nc.tensor.matmul(zps[:], cT[:, it, :], w2[:], start=False,
                             stop=not has_bias)
            if has_bias:
                nc.tensor.matmul(zps[:], ones1[:], brow[:], start=False,
                                 stop=True)
            nc.scalar.activation(zall[:, it, :, :], zps[:], AF.Copy)

        # ---- batched l2norm scales: rr = rsqrt(max(sum(z^2), eps^2))
        sq = pmid.tile([128, NB, 2, F], dt.bfloat16, tag="sq")
        nc.vector.tensor_mul(sq[:], zall[:], zall[:])
        n2b = pmid.tile([128, NB, 2], dt.bfloat16, tag="n2b")
        with nc.allow_low_precision(reason="norm^2 in bf16 is within gate"):
            nc.vector.tensor_reduce(n2b[:], sq[:], axis=mybir.AxisListType.X,
                                    op=mybir.AluOpType.add)
        rr = pmid.tile([128, NB, 2], dt.float32, tag="rr")
        t0 = pmid.tile([128, NB, 2], dt.float32, tag="t0")
        nc.vector.tensor_scalar_max(rr[:], n2b[:], EPS * EPS)
        rri = rr[:].bitcast(dt.int32)
        t0i = t0[:].bitcast(dt.int32)
        nc.vector.tensor_scalar(t0i, rri, 1, None,
                                mybir.AluOpType.logical_shift_right)
        nc.vector.tensor_tensor(t0i, magic[:], t0i, mybir.AluOpType.subtract)
        # y = y * (1.5 - 0.5 * x * y^2), x = clamped n2 (in rr)
        for _ in range(3):
            y2 = pmid.tile([128, NB, 2], dt.float32, tag="y2")
            nc.vector.tensor_tensor(y2[:], t0[:], t0[:], mybir.AluOpType.mult)
            nc.vector.tensor_tensor(y2[:], y2[:], rr[:], mybir.AluOpType.mult)
            nc.vector.tensor_scalar(y2[:], y2[:], -0.5, 1.5,
                                    mybir.AluOpType.mult, mybir.AluOpType.add)
            nc.vector.tensor_tensor(t0[:], t0[:], y2[:], mybir.AluOpType.mult)

        # ---- feat = relu(z*rr) (bf16); ypool likewise; softmax via one Exp
        zn = pmid.tile([128, NB, 2, F], dt.bfloat16, tag="zn")
        rrb = t0[:, :, :, None].broadcast_to([128, NB, 2, F])
        nc.vector.tensor_tensor(zn[:], zall[:], rrb, mybir.AluOpType.mult)
        FA = pmid.tile([128, NB, F], dt.bfloat16, tag="FA")       # feat
        ypall = pmid.tile([128, NB, F], dt.bfloat16, tag="ypall")
        nc.vector.tensor_scalar_max(FA[:], zn[:, :, 0, :], 0.0)
        nc.vector.tensor_scalar_max(ypall[:], zn[:, :, 1, :], 0.0)
        eyall = pmid.tile([128, NB, F], dt.bfloat16, tag="eyall")
        nc.scalar.activation(eyall[:], ypall[:], AF.Exp)
        seall = pmid.tile([128, NB], dt.float32, tag="seall")
        nc.vector.tensor_reduce(seall[:], eyall[:], axis=mybir.AxisListType.X,
                                op=mybir.AluOpType.add)
        rse = pmid.tile([128, NB], dt.float32, tag="rse")
        nc.vector.reciprocal(rse[:], seall[:])
        S = pmid.tile([128, NB, F], dt.bfloat16, tag="S")
        rseb = rse[:, :, None].broadcast_to([128, NB, F])
        nc.vector.tensor_tensor(S[:], eyall[:], rseb, mybir.AluOpType.mult)

        # ---- T = A^T S (same orientation as c); blocks = T^T S
        T = pmid.tile([128, NB, F], dt.bfloat16, tag="T")
        tps = ps_t.tile([128, NB, F], dt.float32, tag="tps")
        for vt in range(NB):
            for j in range(NB):
                nc.tensor.matmul(
                    tps[:, vt, :], A[:, j, ts(vt, 128)], S[:, j, :],
                    start=(j == 0), stop=(j == NB - 1),
                )
        nc.scalar.activation(T[:], tps[:], AF.Copy)

        # ---- h_new = S^T feat ; blocks = T^T S
        ot = pout.tile([128, 2 * F], dt.float32, tag="ot")
        hps = ps_z.tile([128, 2 * F], dt.float32, tag="zps")
        for j in range(NB):
            nc.tensor.matmul(hps[:, 0:F], S[:, j, :], FA[:, j, :],
                             start=(j == 0), stop=(j == NB - 1))
        for j in range(NB):
            nc.tensor.matmul(hps[:, F:], T[:, j, :], S[:, j, :],
                             start=(j == 0), stop=(j == NB - 1))
        nc.scalar.activation(ot[:], hps[:], AF.Copy)
        nc.sync.dma_start(out=P["out_hb"].ap()[g], in_=ot[:])


def _build(Spad, has_bias):
    key = (Spad, has_bias)
    if key in _compiled:
        return _compiled[key]
    nc = bacc.Bacc("TRN2", target_bir_lowering=False, debug=False)
    P = {
        "haug": nc.declare_dram_parameter("haug", [GPC, 128, NB, FP], dt.bfloat16, isOutput=False),
        "ht": nc.declare_dram_parameter("ht", [GPC, 128, n], dt.bfloat16, isOutput=False),
        "sidx": nc.declare_dram_parameter("sidx", [GPC, 128, NB, Spad], dt.int16, isOutput=False),
        "scnt": nc.declare_dram_parameter("scnt", [GPC, 128, NB, Spad], dt.bfloat16, isOutput=False),
        "w1": nc.declare_dram_parameter("w1", [128, 2 * F], dt.bfloat16, isOutput=False),
        "w2": nc.declare_dram_parameter("w2", [128, 2 * F], dt.bfloat16, isOutput=False),
        "brow": nc.declare_dram_parameter("brow", [1, 2 * F], dt.bfloat16, isOutput=False),
        "ident": nc.declare_dram_parameter("ident", [128, 128], dt.bfloat16, isOutput=False),
        "ones1": nc.declare_dram_parameter("ones1", [1, 128], dt.bfloat16, isOutput=False),
        "out_hb": nc.declare_dram_parameter("out_hb", [GPC, 128, 2 * F], dt.float32, isOutput=True),
    }
    with tile.TileContext(nc) as tc, ExitStack() as ctx:
        _emit(ctx, tc, P, Spad, has_bias)
    nc.compile()
    _compiled[key] = nc
    return nc


# ---------------------------------------------------------------- entry

def kernel(h, src_local, dst_local, W_feat, b_feat, W_pool, b_pool,
           _trace=False, _tmpdir=None):
    in_maps, Spad = _host_prep(h, src_local, dst_local,
                               W_feat, b_feat, W_pool, b_pool)
    has_bias = bool(np.any(np.asarray(b_feat)) or np.any(np.asarray(b_pool)))
    nc = _build(Spad, has_bias)
    res = run_bass_kernel_spmd(nc, in_maps, core_ids=list(range(NCORES)),
                               trace=_trace, tmpdir=_tmpdir)
    kernel.last_results = res

    h_new = np.zeros((B * K, F), dtype=np.float32)
    adj_new = np.zeros((B * K, B * K), dtype=np.float32)
    for m in range(NCORES):
        out = np.asarray(res.results[m]["out_hb"])     # [GPC, 128, 256] f32
        for i in range(GPC):
            g = m * GPC + i
            h_new[g * K:(g + 1) * K] = out[i, :, :F]
            adj_new[g * K:(g + 1) * K, g * K:(g + 1) * K] = out[i, :, F:]
    return adj_new, h_new


# revision 12
# speedup vs baseline: 1.5823x; 1.0566x over previous
"""DiffPool batched-graph layer on 8 TRN2 NeuronCores (Bass/Tile).

Strategy: shard the B=64 graphs across 8 cores (8 graphs/core). All segment
ops become dense per-graph matmuls against the graph's adjacency-count matrix
A [n, n], which is built on-chip in SBUF by GPSIMD local_scatter from
host-packed (dedup'd, per-src-node padded) edge lists. A^T comes from PE
transposes. The mean-aggregator degree falls out of the same matmul via an
appended ones-column on h. Final h_new/blocks are per-graph [128, 256] tiles;
the host assembles the block-diagonal adj_new.
"""
from contextlib import ExitStack

import numpy as np
import ml_dtypes

import concourse.bass as bass
import concourse.bacc as bacc
import concourse.mybir as mybir
from concourse import tile
from concourse.bass_utils import run_bass_kernel_spmd

bf16 = ml_dtypes.bfloat16
AF = mybir.ActivationFunctionType
dt = mybir.dt

# Problem constants (hardcoded per harness contract)
B, n, F, K, E = 64, 1024, 128, 128, 16384
NCORES = 8
GPC = B // NCORES       # graphs per core
NB = n // 128           # 128-row blocks per graph
EPS = 1e-12
FP = F + 2              # h_aug free: F features + ones col + pad

_compiled = {}          # Spad -> compiled Bacc


# ---------------------------------------------------------------- host prep

def _pack_graph_edges(src, dst, Spad):
    """One graph's edges -> (idx int16, data bf16 [128, NB, Spad], deg [n] f32).

    Dedup (src,dst) with counts; node u = b*128+p gets its (dst, count/deg[dst])
    list in partition p, block b, padded with -1. The scatter data carries the
    mean-aggregation divide; deg is shipped separately to undo it where the raw
    adjacency is needed (blocks).
    """
    key = src.astype(np.int64) * n + dst.astype(np.int64)
    uniq, counts = np.unique(key, return_counts=True)
    us = (uniq // n).astype(np.int64)
    vs = (uniq % n).astype(np.int64)
    deg = np.bincount(dst.astype(np.int64), minlength=n).astype(np.float32)
    degc = np.maximum(deg, 1.0)
    # slot j within each run of equal us (uniq is sorted so us runs are contiguous)
    starts = np.r_[0, np.flatnonzero(np.diff(us)) + 1]
    run_len = np.diff(np.r_[starts, len(us)])
    j = np.arange(len(us)) - np.repeat(starts, run_len)
    smax = int(j.max()) + 1 if len(j) else 1
    if smax > Spad:
        raise ValueError(f"Spad {Spad} too small, need {smax}")
    idx = np.full((128, NB, Spad), -1, dtype=np.int16)
    cnt = np.zeros((128, NB, Spad), dtype=bf16)
    idx[us % 128, us // 128, j] = vs.astype(np.int16)
    cnt[us % 128, us // 128, j] = (counts / degc[vs]).astype(bf16)
    return idx, cnt, degc


def _host_prep(h, src_local, dst_local, W_feat, b_feat, W_pool, b_pool):
    h = np.asarray(h, dtype=np.float32)
    src_local = np.asarray(src_local)
    dst_local = np.asarray(dst_local)

    # global Spad (compile-time constant), rounded up to multiple of 4
    deg_max = 0
    packs = []
    # first pass to find Spad cheaply: count max multiplicity-run per src node
    for g in range(B):
        key = src_local[g].astype(np.int64) * n + dst_local[g].astype(np.int64)
        uniq = np.unique(key)
        us = uniq // n
        _, c = np.unique(us, return_counts=True)
        deg_max = max(deg_max, int(c.max()))
    Spad = (deg_max + 3) // 4 * 4

    for g in range(B):
        packs.append(_pack_graph_edges(src_local[g], dst_local[g], Spad))

    hb = h.astype(bf16)                       # [N, F] bf16
    w1 = np.concatenate([W_feat[:F], W_pool[:F]], axis=1).astype(bf16)   # [128, 256]
    w2 = np.concatenate([W_feat[F:], W_pool[F:]], axis=1).astype(bf16)   # [128, 256]
    brow = np.concatenate([b_feat, b_pool]).reshape(1, 2 * F).astype(bf16)
    ident = np.eye(128, dtype=bf16)
    ones1 = np.ones((1, 128), dtype=bf16)

    in_maps = []
    for m in range(NCORES):
        gs = range(m * GPC, (m + 1) * GPC)
        hnat = np.zeros((GPC, 128, NB, F), dtype=bf16)
        ht = np.zeros((GPC, 128, n), dtype=bf16)
        sidx = np.zeros((GPC, 128, NB, Spad), dtype=np.int16)
        scnt = np.zeros((GPC, 128, NB, Spad), dtype=bf16)
        degb = np.zeros((GPC, 128, NB), dtype=bf16)
        for i, g in enumerate(gs):
            hg = hb[g * n:(g + 1) * n]                     # [1024, 128] bf16
            # natural layout [p, b, f] for node u = b*128+p
            hnat[i] = hg.reshape(NB, 128, F).transpose(1, 0, 2)
            ht[i] = hg.T                                   # [F, 1024]
            sidx[i], scnt[i], degc = packs[g]
            degb[i] = degc.reshape(NB, 128).T              # [v_lo, vt]
        in_maps.append({
            "hnat": hnat, "ht": ht, "sidx": sidx, "scnt": scnt, "degb": degb,
            "w1": w1, "w2": w2, "brow": brow, "ident": ident, "ones1": ones1,
        })
    return in_maps, Spad


# ---------------------------------------------------------------- device IR

def _emit(ctx: ExitStack, tc: tile.TileContext, P, Spad, has_bias):
    nc = tc.nc
    ts = bass.ts
    H = n // 2   # 512: max moving free dim / PSUM bank

    consts = ctx.enter_context(tc.tile_pool(name="consts", bufs=1))
    pin = ctx.enter_context(tc.tile_pool(name="pin", bufs=3))
    pA = ctx.enter_context(tc.tile_pool(name="pA", bufs=3))
    pmid = ctx.enter_context(tc.tile_pool(name="pmid", bufs=2))
    pout = ctx.enter_context(tc.tile_pool(name="pout", bufs=2))
    ps_c = ctx.enter_context(tc.tile_pool(name="ps_c", bufs=2, space="PSUM"))
    ps_z = ctx.enter_context(tc.tile_pool(name="ps_z", bufs=2, space="PSUM"))
    ps_tt = ctx.enter_context(tc.tile_pool(name="ps_tt", bufs=2, space="PSUM"))
    ps_tn = ctx.enter_context(tc.tile_pool(name="ps_tn", bufs=1, space="PSUM"))

    # constants, loaded once
    w1 = consts.tile([128, 2 * F], dt.bfloat16)
    w2 = consts.tile([128, 2 * F], dt.bfloat16)
    brow = consts.tile([1, 2 * F], dt.bfloat16)
    ident = consts.tile([128, 128], dt.bfloat16)
    ones1 = consts.tile([1, 128], dt.bfloat16)
    magic = consts.tile([128, NB, 2], dt.int32)
    nc.sync.dma_start(out=w1[:], in_=P["w1"].ap())
    nc.sync.dma_start(out=w2[:], in_=P["w2"].ap())
    nc.sync.dma_start(out=brow[:], in_=P["brow"].ap())
    nc.sync.dma_start(out=ident[:], in_=P["ident"].ap())
    nc.sync.dma_start(out=ones1[:], in_=P["ones1"].ap())
    nc.vector.memset(magic[:], 0x5F3759DF)

    for g in range(GPC):
        # ---- loads
        hn = pin.tile([128, NB, F], dt.bfloat16, tag="hn")
        ht = pin.tile([128, n], dt.bfloat16, tag="ht")
        six = pin.tile([128, NB, Spad], dt.int16, tag="six")
        sct = pin.tile([128, NB, Spad], dt.bfloat16, tag="sct")
        deg = pin.tile([128, NB], dt.bfloat16, tag="deg")
        nc.sync.dma_start(out=hn[:], in_=P["hnat"].ap()[g])
        nc.sync.dma_start(out=ht[:], in_=P["ht"].ap()[g])
        nc.sync.dma_start(out=six[:], in_=P["sidx"].ap()[g])
        nc.sync.dma_start(out=sct[:], in_=P["scnt"].ap()[g])
        nc.sync.dma_start(out=deg[:], in_=P["degb"].ap()[g])

        # ---- A build: A[u_lo, u_hi, v] = count(u -> v) / deg(v)
        A = pA.tile([128, NB, n], dt.bfloat16, tag="A")
        for b in range(NB):
            nc.gpsimd.local_scatter(
                A[:, b, :], sct[:, b, :], six[:, b, :],
                channels=128, num_elems=n, num_idxs=Spad,
            )

        # ---- c^T = h^T A_s  (mean aggregation, transposed layout, stationary h)
        cT = pmid.tile([128, n], dt.bfloat16, tag="cT")
        for half in range(2):
            cps = ps_c.tile([128, H], dt.float32, tag="cps")
            for j in range(NB):
                nc.tensor.matmul(
                    cps[:], hn[:, j, :], A[:, j, ts(half, H)],
                    start=(j == 0), stop=(j == NB - 1),
                )
            nc.scalar.activation(cT[:, ts(half, H)], cps[:], AF.Copy)

        # ---- z = [h, c] @ [W_feat | W_pool] (+ b) -> zall (bf16)
        zall = pmid.tile([128, NB, 2, F], dt.bfloat16, tag="zall")
        for it in range(NB):
            zps = ps_z.tile([128, 2 * F], dt.float32, tag="zps")
            nc.tensor.matmul(zps[:], ht[:, ts(it, 128)], w1[:], start=True,
                             stop=False)
            nc.tensor.matmul(zps[:], cT[:, ts(it, 128)], w2[:], start=False,
                             stop=not has_bias)
            if has_bias:
                nc.tensor.matmul(zps[:], ones1[:], brow[:], start=False,
                                 stop=True)
            nc.scalar.activation(zall[:, it, :, :], zps[:], AF.Copy)

        # ---- batched l2norm scales: rr = rsqrt(max(sum(z^2), eps^2))
        sq = pmid.tile([128, NB, 2, F], dt.bfloat16, tag="sq")
        nc.vector.tensor_mul(sq[:], zall[:], zall[:])
        n2b = pmid.tile([128, NB, 2], dt.bfloat16, tag="n2b")
        with nc.allow_low_precision(reason="norm^2 in bf16 is within gate"):
            nc.vector.tensor_reduce(n2b[:], sq[:], axis=mybir.AxisListType.X,
                                    op=mybir.AluOpType.add)
        rr = pmid.tile([128, NB, 2], dt.float32, tag="rr")
        t0 = pmid.tile([128, NB, 2], dt.float32, tag="t0")
        nc.vector.tensor_scalar_max(rr[:], n2b[:], EPS * EPS)
        rri = rr[:].bitcast(dt.int32)
        t0i = t0[:].bitcast(dt.int32)
        nc.vector.tensor_scalar(t0i, rri, 1, None,
                                mybir.AluOpType.logical_shift_right)
        nc.vector.tensor_tensor(t0i, magic[:], t0i, mybir.AluOpType.subtract)
        # y = y * (1.5 - 0.5 * x * y^2), x = clamped n2 (in rr)
        for _ in range(2):
            y2 = pmid.tile([128, NB, 2], dt.float32, tag="y2")
            nc.vector.tensor_tensor(y2[:], t0[:], t0[:], mybir.AluOpType.mult)
            nc.vector.tensor_tensor(y2[:], y2[:], rr[:], mybir.AluOpType.mult)
            nc.vector.tensor_scalar(y2[:], y2[:], -0.5, 1.5,
                                    mybir.AluOpType.mult, mybir.AluOpType.add)
            nc.vector.tensor_tensor(t0[:], t0[:], y2[:], mybir.AluOpType.mult)

        # ---- feat = relu(z*rr) (bf16); ypool likewise; softmax via one Exp
        zn = pmid.tile([128, NB, 2, F], dt.bfloat16, tag="zn")
        rrb = t0[:, :, :, None].broadcast_to([128, NB, 2, F])
        nc.vector.tensor_tensor(zn[:], zall[:], rrb, mybir.AluOpType.mult)
        FA = pmid.tile([128, NB, F], dt.bfloat16, tag="FA")       # feat
        ypall = pmid.tile([128, NB, F], dt.bfloat16, tag="ypall")
        nc.vector.tensor_scalar_max(FA[:], zn[:, :, 0, :], 0.0)
        nc.vector.tensor_scalar_max(ypall[:], zn[:, :, 1, :], 0.0)
        eyall = pmid.tile([128, NB, F], dt.bfloat16, tag="eyall")
        nc.scalar.activation(eyall[:], ypall[:], AF.Exp)
        seall = pmid.tile([128, NB], dt.float32, tag="seall")
        nc.vector.tensor_reduce(seall[:], eyall[:], axis=mybir.AxisListType.X,
                                op=mybir.AluOpType.add)
        rse = pmid.tile([128, NB], dt.float32, tag="rse")
        nc.vector.reciprocal(rse[:], seall[:])
        S = pmid.tile([128, NB, F], dt.bfloat16, tag="S")
        rseb = rse[:, :, None].broadcast_to([128, NB, F])
        nc.vector.tensor_tensor(S[:], eyall[:], rseb, mybir.AluOpType.mult)
        # S_deg = S * deg (undoes the 1/deg folded into A for the blocks matmul)
        Sd = pmid.tile([128, NB, F], dt.bfloat16, tag="Sd")
        degb = deg[:, :, None].broadcast_to([128, NB, F])
        nc.vector.tensor_tensor(Sd[:], S[:], degb, mybir.AluOpType.mult)

        # ---- T^T = S^T A_s (stationary S); transpose tiles to natural layout
        TTs = pmid.tile([128, n], dt.bfloat16, tag="TTs")
        for half in range(2):
            ttps = ps_tt.tile([128, H], dt.float32, tag="ttps")
            for j in range(NB):
                nc.tensor.matmul(
                    ttps[:], S[:, j, :], A[:, j, ts(half, H)],
                    start=(j == 0), stop=(j == NB - 1),
                )
            nc.scalar.activation(TTs[:, ts(half, H)], ttps[:], AF.Copy)
        Tn = pmid.tile([128, NB, F], dt.bfloat16, tag="Tn")
        tnps = ps_tn.tile([128, NB, F], dt.bfloat16, tag="tnps")
        for vt in range(NB):
            nc.tensor.transpose(tnps[:, vt, :], TTs[:, ts(vt, 128)], ident[:])
        nc.scalar.activation(Tn[:], tnps[:], AF.Copy)

        # ---- h_new = S^T feat ; blocks[k, l] = sum_v Tn[v, k] (deg*S)[v, l]
        ot = pout.tile([128, 2 * F], dt.float32, tag="ot")
        hps = ps_z.tile([128, 2 * F], dt.float32, tag="zps")
        for j in range(NB):
            nc.tensor.matmul(hps[:, 0:F], S[:, j, :], FA[:, j, :],
                             start=(j == 0), stop=(j == NB - 1))
        for j in range(NB):
            nc.tensor.matmul(hps[:, F:], Tn[:, j, :], Sd[:, j, :],
                             start=(j == 0), stop=(j == NB - 1))
        nc.scalar.activation(ot[:], hps[:], AF.Copy)
        nc.sync.dma_start(out=P["out_hb"].ap()[g], in_=ot[:])


def _build(Spad, has_bias):
    key = (Spad, has_bias)
    if key in _compiled:
        return _compiled[key]
    nc = bacc.Bacc("TRN2", target_bir_lowering=False, debug=False)
    P = {
        "hnat": nc.declare_dram_parameter("hnat", [GPC, 128, NB, F], dt.bfloat16, isOutput=False),
        "degb": nc.declare_dram_parameter("degb", [GPC, 128, NB], dt.bfloat16, isOutput=False),
        "ht": nc.declare_dram_parameter("ht", [GPC, 128, n], dt.bfloat16, isOutput=False),
        "sidx": nc.declare_dram_parameter("sidx", [GPC, 128, NB, Spad], dt.int16, isOutput=False),
        "scnt": nc.declare_dram_parameter("scnt", [GPC, 128, NB, Spad], dt.bfloat16, isOutput=False),
        "w1": nc.declare_dram_parameter("w1", [128, 2 * F], dt.bfloat16, isOutput=False),
        "w2": nc.declare_dram_parameter("w2", [128, 2 * F], dt.bfloat16, isOutput=False),
        "brow": nc.declare_dram_parameter("brow", [1, 2 * F], dt.bfloat16, isOutput=False),
        "ident": nc.declare_dram_parameter("ident", [128, 128], dt.bfloat16, isOutput=False),
        "ones1": nc.declare_dram_parameter("ones1", [1, 128], dt.bfloat16, isOutput=False),
        "out_hb": nc.declare_dram_parameter("out_hb", [GPC, 128, 2 * F], dt.float32, isOutput=True),
    }
    with tile.TileContext(nc) as tc, ExitStack() as ctx:
        _emit(ctx, tc, P, Spad, has_bias)
    nc.compile()
    _compiled[key] = nc
    return nc


# ---------------------------------------------------------------- entry

def kernel(h, src_local, dst_local, W_feat, b_feat, W_pool, b_pool,
           _trace=False, _tmpdir=None):
    in_maps, Spad = _host_prep(h, src_local, dst_local,
                               W_feat, b_feat, W_pool, b_pool)
    has_bias = bool(np.any(np.asarray(b_feat)) or np.any(np.asarray(b_pool)))
    nc = _build(Spad, has_bias)
    res = run_bass_kernel_spmd(nc, in_maps, core_ids=list(range(NCORES)),
                               trace=_trace, tmpdir=_tmpdir)
    kernel.last_results = res

    h_new = np.zeros((B * K, F), dtype=np.float32)
    adj_new = np.zeros((B * K, B * K), dtype=np.float32)
    for m in range(NCORES):
        out = np.asarray(res.results[m]["out_hb"])     # [GPC, 128, 256] f32
        for i in range(GPC):
            g = m * GPC + i
            h_new[g * K:(g + 1) * K] = out[i, :, :F]
            adj_new[g * K:(g + 1) * K, g * K:(g + 1) * K] = out[i, :, F:]
    return adj_new, h_new


# revision 14
# speedup vs baseline: 1.6870x; 1.0662x over previous
"""DiffPool batched-graph layer on 8 TRN2 NeuronCores (Bass/Tile).

Strategy: shard the B=64 graphs across 8 cores (8 graphs/core). All segment
ops become dense per-graph matmuls against the graph's adjacency-count matrix
A [n, n], which is built on-chip in SBUF by GPSIMD local_scatter from
host-packed (dedup'd, per-src-node padded) edge lists. A^T comes from PE
transposes. The mean-aggregator degree falls out of the same matmul via an
appended ones-column on h. Final h_new/blocks are per-graph [128, 256] tiles;
the host assembles the block-diagonal adj_new.
"""
from contextlib import ExitStack

import numpy as np
import ml_dtypes

import concourse.bass as bass
import concourse.bacc as bacc
import concourse.mybir as mybir
from concourse import tile
from concourse.bass_utils import run_bass_kernel_spmd

bf16 = ml_dtypes.bfloat16
AF = mybir.ActivationFunctionType
dt = mybir.dt

# Problem constants (hardcoded per harness contract)
B, n, F, K, E = 64, 1024, 128, 128, 16384
NCORES = 8
GPC = B // NCORES       # graphs per core
NB = n // 128           # 128-row blocks per graph
EPS = 1e-12
FP = F + 2              # h_aug free: F features + ones col + pad

_compiled = {}          # Spad -> compiled Bacc


# ---------------------------------------------------------------- host prep

def _pack_graph_edges(src, dst, Spad):
    """One graph's edges -> (idx int16, data bf16 [128, NB, Spad], deg [n] f32).

    Dedup (src,dst) with counts; node u = b*128+p gets its (dst, count/deg[dst])
    list in partition p, block b, padded with -1. The scatter data carries the
    mean-aggregation divide; deg is shipped separately to undo it where the raw
    adjacency is needed (blocks).
    """
    key = src.astype(np.int64) * n + dst.astype(np.int64)
    uniq, counts = np.unique(key, return_counts=True)
    us = (uniq // n).astype(np.int64)
    vs = (uniq % n).astype(np.int64)
    deg = np.bincount(dst.astype(np.int64), minlength=n).astype(np.float32)
    degc = np.maximum(deg, 1.0)
    # slot j within each run of equal us (uniq is sorted so us runs are contiguous)
    starts = np.r_[0, np.flatnonzero(np.diff(us)) + 1]
    run_len = np.diff(np.r_[starts, len(us)])
    j = np.arange(len(us)) - np.repeat(starts, run_len)
    smax = int(j.max()) + 1 if len(j) else 1
    if smax > Spad:
        raise ValueError(f"Spad {Spad} too small, need {smax}")
    idx = np.full((128, NB, Spad), -1, dtype=np.int16)
    cnt = np.zeros((128, NB, Spad), dtype=bf16)
    idx[us % 128, us // 128, j] = vs.astype(np.int16)
    cnt[us % 128, us // 128, j] = (counts / degc[vs]).astype(bf16)
    return idx, cnt, degc


def _host_prep(h, src_local, dst_local, W_feat, b_feat, W_pool, b_pool):
    h = np.asarray(h, dtype=np.float32)
    src_local = np.asarray(src_local)
    dst_local = np.asarray(dst_local)

    # global Spad (compile-time constant), rounded up to multiple of 4
    deg_max = 0
    packs = []
    # first pass to find Spad cheaply: count max multiplicity-run per src node
    for g in range(B):
        key = src_local[g].astype(np.int64) * n + dst_local[g].astype(np.int64)
        uniq = np.unique(key)
        us = uniq // n
        _, c = np.unique(us, return_counts=True)
        deg_max = max(deg_max, int(c.max()))
    Spad = (deg_max + 3) // 4 * 4

    for g in range(B):
        packs.append(_pack_graph_edges(src_local[g], dst_local[g], Spad))

    hb = h.astype(bf16)                       # [N, F] bf16
    w1 = np.concatenate([W_feat[:F], W_pool[:F]], axis=1).astype(bf16)   # [128, 256]
    w2 = np.concatenate([W_feat[F:], W_pool[F:]], axis=1).astype(bf16)   # [128, 256]
    brow = np.concatenate([b_feat, b_pool]).reshape(1, 2 * F).astype(bf16)
    ident = np.eye(128, dtype=bf16)
    ones1 = np.ones((1, 128), dtype=bf16)

    in_maps = []
    for m in range(NCORES):
        gs = range(m * GPC, (m + 1) * GPC)
        hnat = np.zeros((GPC, 128, NB, F), dtype=bf16)
        ht = np.zeros((GPC, 128, n), dtype=bf16)
        sidx = np.zeros((GPC, 128, NB, Spad), dtype=np.int16)
        scnt = np.zeros((GPC, 128, NB, Spad), dtype=bf16)
        degb = np.zeros((GPC, 128, NB), dtype=bf16)
        for i, g in enumerate(gs):
            hg = hb[g * n:(g + 1) * n]                     # [1024, 128] bf16
            # natural layout [p, b, f] for node u = b*128+p
            hnat[i] = hg.reshape(NB, 128, F).transpose(1, 0, 2)
            ht[i] = hg.T                                   # [F, 1024]
            sidx[i], scnt[i], degc = packs[g]
            degb[i] = degc.reshape(NB, 128).T              # [v_lo, vt]
        in_maps.append({
            "hnat": hnat, "ht": ht, "sidx": sidx, "scnt": scnt, "degb": degb,
            "w1": w1, "w2": w2, "brow": brow, "ident": ident, "ones1": ones1,
        })
    return in_maps, Spad


# ---------------------------------------------------------------- device IR

def _emit(ctx: ExitStack, tc: tile.TileContext, P, Spad, has_bias):
    nc = tc.nc
    ts = bass.ts
    H = n // 2   # 512: max moving free dim / PSUM bank

    consts = ctx.enter_context(tc.tile_pool(name="consts", bufs=1))
    pin2 = ctx.enter_context(tc.tile_pool(name="pin2", bufs=2))
    pin3 = ctx.enter_context(tc.tile_pool(name="pin3", bufs=3))
    pin4 = ctx.enter_context(tc.tile_pool(name="pin4", bufs=4))
    pA = ctx.enter_context(tc.tile_pool(name="pA", bufs=4))
    pm2 = ctx.enter_context(tc.tile_pool(name="pm2", bufs=2))
    pm3 = ctx.enter_context(tc.tile_pool(name="pm3", bufs=3))
    pout = ctx.enter_context(tc.tile_pool(name="pout", bufs=2))
    ps_c = ctx.enter_context(tc.tile_pool(name="ps_c", bufs=2, space="PSUM"))
    ps_z = ctx.enter_context(tc.tile_pool(name="ps_z", bufs=3, space="PSUM"))
    ps_tt = ctx.enter_context(tc.tile_pool(name="ps_tt", bufs=2, space="PSUM"))
    ps_tn = ctx.enter_context(tc.tile_pool(name="ps_tn", bufs=1, space="PSUM"))

    # constants, loaded once
    w1 = consts.tile([128, 2 * F], dt.bfloat16)
    w2 = consts.tile([128, 2 * F], dt.bfloat16)
    brow = consts.tile([1, 2 * F], dt.bfloat16)
    ident = consts.tile([128, 128], dt.bfloat16)
    ones1 = consts.tile([1, 128], dt.bfloat16)
    magic = consts.tile([128, NB, 2], dt.int32)
    nc.sync.dma_start(out=w1[:], in_=P["w1"].ap())
    nc.sync.dma_start(out=w2[:], in_=P["w2"].ap())
    nc.sync.dma_start(out=brow[:], in_=P["brow"].ap())
    nc.sync.dma_start(out=ident[:], in_=P["ident"].ap())
    nc.sync.dma_start(out=ones1[:], in_=P["ones1"].ap())
    nc.vector.memset(magic[:], 0x5F3759DF)

    st = [dict() for _ in range(GPC)]   # per-graph live tiles

    def s0_load(g):
        d = st[g]
        d["hn"] = pin3.tile([128, NB, F], dt.bfloat16, tag="hn", name="hn")
        d["ht"] = pin4.tile([128, n], dt.bfloat16, tag="ht", name="ht")
        d["six"] = pin2.tile([128, NB, Spad], dt.int16, tag="six", name="six")
        d["sct"] = pin2.tile([128, NB, Spad], dt.bfloat16, tag="sct", name="sct")
        d["deg"] = pin4.tile([128, NB], dt.bfloat16, tag="deg", name="deg")
        nc.sync.dma_start(out=d["hn"][:], in_=P["hnat"].ap()[g])
        nc.sync.dma_start(out=d["ht"][:], in_=P["ht"].ap()[g])
        nc.sync.dma_start(out=d["six"][:], in_=P["sidx"].ap()[g])
        nc.sync.dma_start(out=d["sct"][:], in_=P["scnt"].ap()[g])
        nc.sync.dma_start(out=d["deg"][:], in_=P["degb"].ap()[g])

    def s1_scatter(g):
        d = st[g]
        d["A"] = pA.tile([128, NB, n], dt.bfloat16, tag="A", name="A")
        for b in range(NB):
            nc.gpsimd.local_scatter(
                d["A"][:, b, :], d["sct"][:, b, :], d["six"][:, b, :],
                channels=128, num_elems=n, num_idxs=Spad,
            )

    def s2_aggregate(g):
        d = st[g]
        d["cT"] = pm2.tile([128, n], dt.bfloat16, tag="cT", name="cT")
        for half in range(2):
            cps = ps_c.tile([128, H], dt.float32, tag="cps")
            for j in range(NB):
                nc.tensor.matmul(
                    cps[:], d["hn"][:, j, :], d["A"][:, j, ts(half, H)],
                    start=(j == 0), stop=(j == NB - 1),
                )
            nc.scalar.activation(d["cT"][:, ts(half, H)], cps[:], AF.Copy)

    def s3_sage(g):
        d = st[g]
        zall = pm2.tile([128, NB, 2, F], dt.bfloat16, tag="zall")
        for it in range(NB):
            zps = ps_z.tile([128, 2 * F], dt.float32, tag="zps")
            nc.tensor.matmul(zps[:], d["ht"][:, ts(it, 128)], w1[:],
                             start=True, stop=False)
            nc.tensor.matmul(zps[:], d["cT"][:, ts(it, 128)], w2[:],
                             start=False, stop=not has_bias)
            if has_bias:
                nc.tensor.matmul(zps[:], ones1[:], brow[:], start=False,
                                 stop=True)
            nc.scalar.activation(zall[:, it, :, :], zps[:], AF.Copy)

        sq = pm2.tile([128, NB, 2, F], dt.bfloat16, tag="sq")
        nc.vector.tensor_mul(sq[:], zall[:], zall[:])
        n2b = pm2.tile([128, NB, 2], dt.bfloat16, tag="n2b")
        with nc.allow_low_precision(reason="norm^2 in bf16 is within gate"):
            nc.vector.tensor_reduce(n2b[:], sq[:], axis=mybir.AxisListType.X,
                                    op=mybir.AluOpType.add)
        rr = pm2.tile([128, NB, 2], dt.float32, tag="rr")
        t0 = pm2.tile([128, NB, 2], dt.float32, tag="t0")
        nc.vector.tensor_scalar_max(rr[:], n2b[:], EPS * EPS)
        rri = rr[:].bitcast(dt.int32)
        t0i = t0[:].bitcast(dt.int32)
        nc.vector.tensor_scalar(t0i, rri, 1, None,
                                mybir.AluOpType.logical_shift_right)
        nc.vector.tensor_tensor(t0i, magic[:], t0i, mybir.AluOpType.subtract)
        for _ in range(2):
            y2 = pm2.tile([128, NB, 2], dt.float32, tag="y2")
            nc.vector.tensor_tensor(y2[:], t0[:], t0[:], mybir.AluOpType.mult)
            nc.vector.tensor_tensor(y2[:], y2[:], rr[:], mybir.AluOpType.mult)
            nc.vector.tensor_scalar(y2[:], y2[:], -0.5, 1.5,
                                    mybir.AluOpType.mult, mybir.AluOpType.add)
            nc.vector.tensor_tensor(t0[:], t0[:], y2[:], mybir.AluOpType.mult)

        zn = pm2.tile([128, NB, 2, F], dt.bfloat16, tag="zn")
        rrb = t0[:, :, :, None].broadcast_to([128, NB, 2, F])
        nc.vector.tensor_tensor(zn[:], zall[:], rrb, mybir.AluOpType.mult)
        d["FA"] = pm3.tile([128, NB, F], dt.bfloat16, tag="FA", name="FA")
        ypall = pm2.tile([128, NB, F], dt.bfloat16, tag="ypall")
        nc.vector.tensor_scalar_max(d["FA"][:], zn[:, :, 0, :], 0.0)
        nc.vector.tensor_scalar_max(ypall[:], zn[:, :, 1, :], 0.0)
        eyall = pm2.tile([128, NB, F], dt.bfloat16, tag="eyall")
        nc.scalar.activation(eyall[:], ypall[:], AF.Exp)
        seall = pm2.tile([128, NB], dt.float32, tag="seall")
        nc.vector.tensor_reduce(seall[:], eyall[:], axis=mybir.AxisListType.X,
                                op=mybir.AluOpType.add)
        rse = pm2.tile([128, NB], dt.float32, tag="rse")
        nc.vector.reciprocal(rse[:], seall[:])
        d["S"] = pm3.tile([128, NB, F], dt.bfloat16, tag="S", name="S")
        rseb = rse[:, :, None].broadcast_to([128, NB, F])
        nc.vector.tensor_tensor(d["S"][:], eyall[:], rseb,
                                mybir.AluOpType.mult)
        d["Sd"] = pm3.tile([128, NB, F], dt.bfloat16, tag="Sd", name="Sd")
        degb = d["deg"][:, :, None].broadcast_to([128, NB, F])
        nc.vector.tensor_tensor(d["Sd"][:], d["S"][:], degb,
                                mybir.AluOpType.mult)

    def s4_pool(g):
        d = st[g]
        TTs = pm2.tile([128, n], dt.bfloat16, tag="TTs")
        for half in range(2):
            ttps = ps_tt.tile([128, H], dt.float32, tag="ttps")
            for j in range(NB):
                nc.tensor.matmul(
                    ttps[:], d["S"][:, j, :], d["A"][:, j, ts(half, H)],
                    start=(j == 0), stop=(j == NB - 1),
                )
            nc.scalar.activation(TTs[:, ts(half, H)], ttps[:], AF.Copy)
        Tn = pm2.tile([128, NB, F], dt.bfloat16, tag="Tn")
        tnps = ps_tn.tile([128, NB, F], dt.bfloat16, tag="tnps")
        for vt in range(NB):
            nc.tensor.transpose(tnps[:, vt, :], TTs[:, ts(vt, 128)], ident[:])
        nc.scalar.activation(Tn[:], tnps[:], AF.Copy)

        ot = pout.tile([128, 2 * F], dt.float32, tag="ot")
        hps = ps_z.tile([128, 2 * F], dt.float32, tag="zps")
        for j in range(NB):
            nc.tensor.matmul(hps[:, 0:F], d["S"][:, j, :], d["FA"][:, j, :],
                             start=(j == 0), stop=(j == NB - 1))
        for j in range(NB):
            nc.tensor.matmul(hps[:, F:], Tn[:, j, :], d["Sd"][:, j, :],
                             start=(j == 0), stop=(j == NB - 1))
        nc.scalar.activation(ot[:], hps[:], AF.Copy)
        nc.sync.dma_start(out=P["out_hb"].ap()[g], in_=ot[:])
        st[g] = {}

    stages = [s0_load, s1_scatter, s2_aggregate, s3_sage, s4_pool]
    NS = len(stages)
    for tick in range(GPC + NS - 1):
        # deepest stage first within a tick
        for si in reversed(range(NS)):
            g = tick - si
            if 0 <= g < GPC:
                stages[si](g)


def _build(Spad, has_bias):
    key = (Spad, has_bias)
    if key in _compiled:
        return _compiled[key]
    nc = bacc.Bacc("TRN2", target_bir_lowering=False, debug=False)
    P = {
        "hnat": nc.declare_dram_parameter("hnat", [GPC, 128, NB, F], dt.bfloat16, isOutput=False),
        "degb": nc.declare_dram_parameter("degb", [GPC, 128, NB], dt.bfloat16, isOutput=False),
        "ht": nc.declare_dram_parameter("ht", [GPC, 128, n], dt.bfloat16, isOutput=False),
        "sidx": nc.declare_dram_parameter("sidx", [GPC, 128, NB, Spad], dt.int16, isOutput=False),
        "scnt": nc.declare_dram_parameter("scnt", [GPC, 128, NB, Spad], dt.bfloat16, isOutput=False),
        "w1": nc.declare_dram_parameter("w1", [128, 2 * F], dt.bfloat16, isOutput=False),
        "w2": nc.declare_dram_parameter("w2", [128, 2 * F], dt.bfloat16, isOutput=False),
        "brow": nc.declare_dram_parameter("brow", [1, 2 * F], dt.bfloat16, isOutput=False),
        "ident": nc.declare_dram_parameter("ident", [128, 128], dt.bfloat16, isOutput=False),
        "ones1": nc.declare_dram_parameter("ones1", [1, 128], dt.bfloat16, isOutput=False),
        "out_hb": nc.declare_dram_parameter("out_hb", [GPC, 128, 2 * F], dt.float32, isOutput=True),
    }
    with tile.TileContext(nc) as tc, ExitStack() as ctx:
        _emit(ctx, tc, P, Spad, has_bias)
    nc.compile()
    _compiled[key] = nc
    return nc


# ---------------------------------------------------------------- entry

def kernel(h, src_local, dst_local, W_feat, b_feat, W_pool, b_pool,
           _trace=False, _tmpdir=None):
    in_maps, Spad = _host_prep(h, src_local, dst_local,
                               W_feat, b_feat, W_pool, b_pool)
    has_bias = bool(np.any(np.asarray(b_feat)) or np.any(np.asarray(b_pool)))
    nc = _build(Spad, has_bias)
    res = run_bass_kernel_spmd(nc, in_maps, core_ids=list(range(NCORES)),
                               trace=_trace, tmpdir=_tmpdir)
    kernel.last_results = res

    h_new = np.zeros((B * K, F), dtype=np.float32)
    adj_new = np.zeros((B * K, B * K), dtype=np.float32)
    for m in range(NCORES):
        out = np.asarray(res.results[m]["out_hb"])     # [GPC, 128, 256] f32
        for i in range(GPC):
            g = m * GPC + i
            h_new[g * K:(g + 1) * K] = out[i, :, :F]
            adj_new[g * K:(g + 1) * K, g * K:(g + 1) * K] = out[i, :, F:]
    return adj_new, h_new
